# revision 19
# baseline (speedup 1.0000x reference)
"""DenseAtt pairwise-MLP attention kernel for 8x Trainium2 NeuronCores.

Reference computation (N=1024, D=64, WIDTH=64, HEADS=4, ALPHA=0.2):
    hi = x @ W1a.T ; hj = x @ W1b.T
    h  = lrelu(hi[:,None,:] + hj[None,:,:] + b1)     # [n, n, 64]
    h  = lrelu(h @ W2.T + b2)                        # [n, n, 64]
    s  = lrelu(h @ W3.T + b3)                        # [n, n, 4]
    attn = softmax(s.reshape(4, n, n), axis=-1)      # C-order reshape quirk

The C-order reshape means output row R = 4*i + Q (Q = j>>8) has col
c = 4*(j%256) + h, softmax-normalized over the 1024 cols.  Rows
[512k, 512k+512) depend only on i in [128k, 128k+128) -> shard i across
8 cores, no collectives.

Per-core dataflow (width in partitions, j in free):
  - hjT [64,1024] block-packed to [128,512] (rows 0:64 j<512, 64:128
    j>=512).  ciT[:,i] = W1a@x_i + b1 duplicated to both halves.
  - per i: ACT lrelu(hjT + ci bias) -> r1 [128,512] bf16; blockdiag-W2
    matmul -> w2b PSUM slice; per 4 i: one ACT lrelu(+b2) [128,2048]
    -> r2b; blockdiag-W3 matmuls (M=32, col tile_position 32*i4) pack
    4 i into s4b PSUM [128 rows = 32*i4 + 4*fhat + h, 512 = 256q+jw].
  - per 2 g (8 i): ACT lrelu(+b3) [128,1024] -> EL big tile [128,4096]
    per 32-i group; free = 512*g + 256*q + jw.
  - T1 (DVE 32x32 stream transpose, plain [128,4096]):
      TA[32*i4 + jl][512g + 256q + 32jh + (4f+h)] = EL[32*i4 + 4f+h][...jw=32jh+jl]
  - T2 (DVE transpose on strided views) pulls (g,q,f) into partitions:
      OUT[32*i4 + 4g+2q+f][128jh + 4jl + h]  ==  row (i, Q) dense, cols
      in final order 4*jw + h.
  - exp ACT with accum_out -> row sums Z free; reciprocal; per-partition
    scale -> OUTN; one DMA per 32-i group writes 128 x 4KB contiguous
    HBM rows.
"""

import numpy as np
from contextlib import ExitStack

N, D, HEADS, WIDTH, ALPHA = 1024, 64, 4, 64, 0.2
NCORES = 8
IPC = N // NCORES  # 128 i-rows per core

_PROG_CACHE = {}

CB_COLS = 1024 + IPC + 64 + 64 + 1 + 1 + 1  # xT xTi W1aT W1bT b1p b2p b3pat


def _build_program(n_i=IPC, act_dt_name="bfloat16"):
    import sys
    if '/opt/trn_rl_repo' not in sys.path:
        sys.path.insert(0, '/opt/trn_rl_repo')
    import concourse.bass as bass
    import concourse.bacc as bacc
    import concourse.tile as tile
    from concourse import mybir

    f32 = mybir.dt.float32
    act_dt = getattr(mybir.dt, act_dt_name)
    AF = mybir.ActivationFunctionType
    NG = n_i // 32          # 32-i groups
    assert n_i % 32 == 0

    nc = bacc.Bacc("TRN2", target_bir_lowering=False)
    cb_d = nc.declare_dram_parameter("cb", [128, CB_COLS], f32, isOutput=False)
    cbh_d = nc.declare_dram_parameter("cbh", [128, 160], act_dt, isOutput=False)
    out_d = nc.declare_dram_parameter("out", [4 * n_i, N], f32, isOutput=True)

    with ExitStack() as ctx:
        tc = ctx.enter_context(tile.TileContext(nc))
        cp = ctx.enter_context(tc.tile_pool(name="consts", bufs=1))
        rp = ctx.enter_context(tc.tile_pool(name="work", bufs=3))
        gp = ctx.enter_context(tc.tile_pool(name="groups", bufs=2))
        pw = ctx.enter_context(tc.tile_pool(name="pw", bufs=1, space="PSUM"))
        ps4 = ctx.enter_context(tc.tile_pool(name="ps4", bufs=2, space="PSUM"))

        # ---- load all constants with ONE DMA each ----
        cb = cp.tile([128, CB_COLS], f32)
        nc.sync.dma_start(cb[:], cb_d[:])
        cbh = cp.tile([128, 160], act_dt)
        nc.sync.dma_start(cbh[:], cbh_d[:])
        W2bdh = cbh[:, 0:128]
        W3bdh = cbh[:, 128:160]
        o = 0
        xT = cb[0:64, o:o + N]; o += N
        xTi = cb[0:64, o:o + n_i]; o += n_i
        W1aT = cb[0:64, o:o + 64]; o += 64
        W1bT = cb[0:64, o:o + 64]; o += 64
        b1p = cb[:, o:o + 1]; o += 1
        b2p = cb[:, o:o + 1]; o += 1
        b3pat = cb[:, o:o + 1]; o += 1

        # ---- precompute hjT packed + ciT (reuses main-loop PSUM bufs) ----
        hj_ps = pw.tile([128, 2048], f32, tag="w2b")
        nc.tensor.matmul(hj_ps[0:64, 0:512], W1bT, xT[:, 0:512],
                         start=True, stop=True)
        nc.tensor.matmul(hj_ps[64:128, 0:512], W1bT, xT[:, 512:1024],
                         start=True, stop=True, tile_position=(0, 64))
        hjT_p = cp.tile([128, 512], act_dt)
        nc.scalar.copy(hjT_p[:], hj_ps[:, 0:512])

        ci_ps = ps4.tile([128, 1024], f32, tag="s4b")
        nc.tensor.matmul(ci_ps[0:64, 0:n_i], W1aT, xTi,
                         start=True, stop=True)
        nc.tensor.matmul(ci_ps[64:128, 0:n_i], W1aT, xTi,
                         start=True, stop=True, tile_position=(0, 64))
        ciT_p = cp.tile([128, n_i], f32)
        nc.vector.tensor_scalar(ciT_p[:], ci_ps[:, 0:n_i], b1p, None,
                                op0=bass.mybir.AluOpType.add)

        # HBM row view: row = 128*G + 16*g + 4*i4 + 2*f + q, iterated in
        # partition order p = 32*i4 + 16*f + 2*g + q.
        hbmv = out_d.rearrange("(G gg ii ff qq) c -> G ii ff gg qq c",
                               gg=8, ii=4, ff=2, qq=2)

        # ---- main loop over 32-i groups ----
        for G in range(NG):
            EL = gp.tile([128, 4096], f32, tag="EL")
            for g in range(8):
                w2b = pw.tile([128, 2048], f32, tag="w2b")
                r1p = rp.tile([128, 2048], act_dt, tag="r1p")
                for i4 in range(4):
                    ip = 32 * G + 4 * g + i4
                    nc.gpsimd.tensor_scalar(r1p[:, 512 * i4:512 * (i4 + 1)],
                                            hjT_p[:], ciT_p[:, ip:ip + 1], None,
                                            op0=bass.mybir.AluOpType.add)
                r1b = rp.tile([128, 2048], act_dt, tag="r1b")
                nc.vector.scalar_tensor_tensor(r1b[:], r1p[:], ALPHA, r1p[:],
                                               op0=bass.mybir.AluOpType.mult,
                                               op1=bass.mybir.AluOpType.max)
                for i4 in range(4):
                    nc.tensor.matmul(w2b[:, 512 * i4:512 * (i4 + 1)], W2bdh,
                                     r1b[:, 512 * i4:512 * (i4 + 1)],
                                     start=True, stop=True)
                r2b = rp.tile([128, 2048], act_dt, tag="r2b")
                nc.scalar.activation(r2b[:], w2b[:], AF.Prelu, bias=b2p,
                                     scale=1.0, alpha=ALPHA)
                if g % 2 == 0:
                    s4b = ps4.tile([128, 1024], f32, tag="s4b")
                for i4 in range(4):
                    nc.tensor.matmul(
                        s4b[32 * i4:32 * i4 + 32, 512 * (g % 2):512 * (g % 2 + 1)],
                        W3bdh, r2b[:, 512 * i4:512 * (i4 + 1)],
                        start=True, stop=True, tile_position=(0, 32 * i4))
                if g % 2 == 1:
                    nc.scalar.activation(EL[:, 1024 * (g // 2):1024 * (g // 2 + 1)],
                                         s4b[:], AF.Prelu, bias=b3pat,
                                         scale=1.0, alpha=ALPHA)
            # T1: swap partition-low5 (2h+f, W3 row order) with jl; dst is
            # the bit-field layout  512*jh + 128*sU + 32*h + 16*f + 2*g + q
            TA = gp.tile([128, 4096], f32, tag="TA")
            t1_in = EL[:].rearrange("p (gq jh jl) -> p gq jh jl",
                                    gq=16, jh=8, jl=32)
            t1_out = TA[:].rearrange("p (jh sl gq) -> p gq jh sl",
                                     jh=8, sl=32, gq=16)
            nc.vector.transpose(t1_out, t1_in)
            # T2: pull (f, g, q) into partitions, jl back to free:
            # OUTp[32*i4 + 16f + 2g + q][128*jh + 4*jl + h]
            OUTp = gp.tile([128, 1024], f32, tag="OUTp")
            t2_in = TA[:].rearrange("p (jh sU h f gq) -> p jh h sU (f gq)",
                                    jh=8, sU=4, h=4, f=2, gq=16)[:, :, :, 0]
            t2_out = OUTp[:].rearrange("p (jh jl h) -> p jh h jl",
                                       jh=8, jl=32, h=4)
            nc.vector.transpose(t2_out, t2_in)
            # exp + row sums (free accumulate), normalize
            EX = gp.tile([128, 1024], f32, tag="EX")
            Z = gp.tile([128, 1], f32, tag="Z")
            nc.scalar.activation(EX[:], OUTp[:], AF.Exp, accum_out=Z[:])
            rz = gp.tile([128, 1], f32, tag="rz")
            nc.vector.reciprocal(rz[:], Z[:])
            OUTN = gp.tile([128, 1024], f32, tag="OUTN")
            nc.vector.tensor_scalar(OUTN[:], EX[:], rz[:], None,
                                    op0=bass.mybir.AluOpType.mult)
            for ii in range(4):
                for ff in range(2):
                    p0 = 32 * ii + 16 * ff
                    nc.sync.dma_start(hbmv[G, ii, ff], OUTN[p0:p0 + 16, :])
    nc.compile()
    return nc


def _host_inputs(x, W1, b1, W2, b2, W3, b3, core, n_i=IPC, act_dt=None):
    import ml_dtypes
    if act_dt is None:
        act_dt = ml_dtypes.bfloat16
    W1a, W1b = W1[:, :D], W1[:, D:]
    xT = np.ascontiguousarray(x.T).astype(np.float32)
    i0 = core * n_i
    W2bd = np.zeros((128, 128), np.float32)
    W2bd[:64, :64] = W2.T
    W2bd[64:, 64:] = W2.T
    W3bd = np.zeros((128, 32), np.float32)
    W3bd[:64, 0:8:2] = W3.T   # half0 -> rows m = 2h
    W3bd[64:, 1:8:2] = W3.T   # half1 -> rows m = 2h + 1
    b3pat = np.asarray([b3[((p % 32) // 2) % 4] for p in range(128)],
                       np.float32)
    cb = np.zeros((128, CB_COLS), np.float32)
    o = 0
    cb[0:64, o:o + 1024] = xT; o += 1024
    cb[0:64, o:o + n_i] = xT[:, i0:i0 + n_i]; o += n_i
    cb[0:64, o:o + 64] = W1a.T; o += 64
    cb[0:64, o:o + 64] = W1b.T; o += 64
    cb[:, o] = np.concatenate([b1, b1]); o += 1
    cb[:, o] = np.concatenate([b2, b2]); o += 1
    cb[:, o] = b3pat; o += 1
    cbh = np.zeros((128, 160), np.float32)
    cbh[:, 0:128] = W2bd
    cbh[:, 128:160] = W3bd
    return {"cb": cb, "cbh": cbh.astype(act_dt)}


def kernel(x, W1, b1, W2, b2, W3, b3):
    import sys
    if '/opt/trn_rl_repo' not in sys.path:
        sys.path.insert(0, '/opt/trn_rl_repo')
    from concourse.bass_utils import run_bass_kernel_spmd

    key = (IPC, "bfloat16")
    if key not in _PROG_CACHE:
        _PROG_CACHE[key] = _build_program(*key)
    nc = _PROG_CACHE[key]

    x = np.asarray(x, np.float32)
    in_maps = [
        _host_inputs(x, np.asarray(W1, np.float32), np.asarray(b1, np.float32),
                     np.asarray(W2, np.float32), np.asarray(b2, np.float32),
                     np.asarray(W3, np.float32), np.asarray(b3, np.float32), k)
        for k in range(NCORES)
    ]
    res = run_bass_kernel_spmd(nc, in_maps, list(range(NCORES)))
    rows = np.concatenate([res.results[k]["out"] for k in range(NCORES)], axis=0)
    return rows.reshape(HEADS, N, N)


# revision 20
# speedup vs baseline: 5.4830x; 5.4830x over previous
"""DenseAtt pairwise-MLP attention kernel for 8x Trainium2 NeuronCores.

Reference computation (N=1024, D=64, WIDTH=64, HEADS=4, ALPHA=0.2):
    hi = x @ W1a.T ; hj = x @ W1b.T
    h  = lrelu(hi[:,None,:] + hj[None,:,:] + b1)     # [n, n, 64]
    h  = lrelu(h @ W2.T + b2)                        # [n, n, 64]
    s  = lrelu(h @ W3.T + b3)                        # [n, n, 4]
    attn = softmax(s.reshape(4, n, n), axis=-1)      # C-order reshape quirk

The C-order reshape means output row R = 4*i + Q (Q = j>>8) has col
c = 4*(j%256) + h, softmax-normalized over the 1024 cols.  Rows
[512k, 512k+512) depend only on i in [128k, 128k+128) -> shard i across
8 cores, no collectives.

Per-core dataflow (width in partitions, j in free):
  - hjT [64,1024] block-packed to [128,512] (rows 0:64 j<512, 64:128
    j>=512).  ciT[:,i] = W1a@x_i + b1 duplicated to both halves.
  - per i: ACT lrelu(hjT + ci bias) -> r1 [128,512] bf16; blockdiag-W2
    matmul -> w2b PSUM slice; per 4 i: one ACT lrelu(+b2) [128,2048]
    -> r2b; blockdiag-W3 matmuls (M=32, col tile_position 32*i4) pack
    4 i into s4b PSUM [128 rows = 32*i4 + 4*fhat + h, 512 = 256q+jw].
  - per 2 g (8 i): ACT lrelu(+b3) [128,1024] -> EL big tile [128,4096]
    per 32-i group; free = 512*g + 256*q + jw.
  - T1 (DVE 32x32 stream transpose, plain [128,4096]):
      TA[32*i4 + jl][512g + 256q + 32jh + (4f+h)] = EL[32*i4 + 4f+h][...jw=32jh+jl]
  - T2 (DVE transpose on strided views) pulls (g,q,f) into partitions:
      OUT[32*i4 + 4g+2q+f][128jh + 4jl + h]  ==  row (i, Q) dense, cols
      in final order 4*jw + h.
  - exp ACT with accum_out -> row sums Z free; reciprocal; per-partition
    scale -> OUTN; one DMA per 32-i group writes 128 x 4KB contiguous
    HBM rows.
"""

import numpy as np
from contextlib import ExitStack

N, D, HEADS, WIDTH, ALPHA = 1024, 64, 4, 64, 0.2
NCORES = 8
IPC = N // NCORES  # 128 i-rows per core

_PROG_CACHE = {}

CB_COLS = 1024 + IPC + 64 + 64 + 1 + 1 + 1  # xT xTi W1aT W1bT b1p b2p b3pat


def _build_program(n_i=IPC, act_dt_name="bfloat16"):
    import sys
    if '/opt/trn_rl_repo' not in sys.path:
        sys.path.insert(0, '/opt/trn_rl_repo')
    import concourse.bass as bass
    import concourse.bacc as bacc
    import concourse.tile as tile
    from concourse import mybir

    f32 = mybir.dt.float32
    act_dt = getattr(mybir.dt, act_dt_name)
    AF = mybir.ActivationFunctionType
    NG = n_i // 32          # 32-i groups
    assert n_i % 32 == 0

    nc = bacc.Bacc("TRN2", target_bir_lowering=False)
    cb_d = nc.declare_dram_parameter("cb", [128, CB_COLS], f32, isOutput=False)
    cbh_d = nc.declare_dram_parameter("cbh", [128, 160], act_dt, isOutput=False)
    out_d = nc.declare_dram_parameter("out", [4 * n_i, N], f32, isOutput=True)

    with ExitStack() as ctx:
        tc = ctx.enter_context(tile.TileContext(nc))
        cp = ctx.enter_context(tc.tile_pool(name="consts", bufs=1))
        rp = ctx.enter_context(tc.tile_pool(name="work", bufs=3))
        gp = ctx.enter_context(tc.tile_pool(name="groups", bufs=2))
        pw = ctx.enter_context(tc.tile_pool(name="pw", bufs=1, space="PSUM"))
        ps4 = ctx.enter_context(tc.tile_pool(name="ps4", bufs=2, space="PSUM"))

        # ---- load all constants with ONE DMA each ----
        cb = cp.tile([128, CB_COLS], f32)
        nc.sync.dma_start(cb[:], cb_d[:])
        cbh = cp.tile([128, 160], act_dt)
        nc.sync.dma_start(cbh[:], cbh_d[:])
        W2bdh = cbh[:, 0:128]
        W3bdh = cbh[:, 128:160]
        o = 0
        xT = cb[0:64, o:o + N]; o += N
        xTi = cb[0:64, o:o + n_i]; o += n_i
        W1aT = cb[0:64, o:o + 64]; o += 64
        W1bT = cb[0:64, o:o + 64]; o += 64
        b1p = cb[:, o:o + 1]; o += 1
        b2p = cb[:, o:o + 1]; o += 1
        b3pat = cb[:, o:o + 1]; o += 1

        # ---- precompute hjT packed + ciT (reuses main-loop PSUM bufs) ----
        hj_ps = pw.tile([128, 2048], f32, tag="w2b")
        nc.tensor.matmul(hj_ps[0:64, 0:512], W1bT, xT[:, 0:512],
                         start=True, stop=True)
        nc.tensor.matmul(hj_ps[64:128, 0:512], W1bT, xT[:, 512:1024],
                         start=True, stop=True, tile_position=(0, 64))
        hjT_p = cp.tile([128, 512], act_dt)
        nc.scalar.copy(hjT_p[:], hj_ps[:, 0:512])

        ci_ps = ps4.tile([128, 1024], f32, tag="s4b")
        nc.tensor.matmul(ci_ps[0:64, 0:n_i], W1aT, xTi,
                         start=True, stop=True)
        nc.tensor.matmul(ci_ps[64:128, 0:n_i], W1aT, xTi,
                         start=True, stop=True, tile_position=(0, 64))
        ciT_p = cp.tile([128, n_i], f32)
        nc.vector.tensor_scalar(ciT_p[:], ci_ps[:, 0:n_i], b1p, None,
                                op0=bass.mybir.AluOpType.add)

        # HBM row view: row = 128*G + 16*g + 4*i4 + 2*f + q, iterated in
        # partition order p = 32*i4 + 16*f + 2*g + q.
        hbmv = out_d.rearrange("(G gg ii ff qq) c -> G ii ff gg qq c",
                               gg=8, ii=4, ff=2, qq=2)

        # ---- main loop over 32-i groups ----
        for G in range(NG):
            EL = gp.tile([128, 4096], f32, tag="EL")
            for g in range(8):
                w2b = pw.tile([128, 2048], f32, tag="w2b")
                r1p = rp.tile([128, 2048], act_dt, tag="r1p")
                for i4 in range(4):
                    ip = 32 * G + 4 * g + i4
                    nc.vector.tensor_scalar(r1p[:, 512 * i4:512 * (i4 + 1)],
                                            hjT_p[:], ciT_p[:, ip:ip + 1], None,
                                            op0=bass.mybir.AluOpType.add)
                r1b = rp.tile([128, 2048], act_dt, tag="r1b")
                nc.vector.scalar_tensor_tensor(r1b[:], r1p[:], ALPHA, r1p[:],
                                               op0=bass.mybir.AluOpType.mult,
                                               op1=bass.mybir.AluOpType.max)
                for i4 in range(4):
                    nc.tensor.matmul(w2b[:, 512 * i4:512 * (i4 + 1)], W2bdh,
                                     r1b[:, 512 * i4:512 * (i4 + 1)],
                                     start=True, stop=True)
                r2b = rp.tile([128, 2048], act_dt, tag="r2b")
                nc.scalar.activation(r2b[:], w2b[:], AF.Prelu, bias=b2p,
                                     scale=1.0, alpha=ALPHA)
                if g % 2 == 0:
                    s4b = ps4.tile([128, 1024], f32, tag="s4b")
                for i4 in range(4):
                    nc.tensor.matmul(
                        s4b[32 * i4:32 * i4 + 32, 512 * (g % 2):512 * (g % 2 + 1)],
                        W3bdh, r2b[:, 512 * i4:512 * (i4 + 1)],
                        start=True, stop=True, tile_position=(0, 32 * i4))
                if g % 2 == 1:
                    nc.scalar.activation(EL[:, 1024 * (g // 2):1024 * (g // 2 + 1)],
                                         s4b[:], AF.Prelu, bias=b3pat,
                                         scale=1.0, alpha=ALPHA)
            # T1: swap partition-low5 (2h+f, W3 row order) with jl; dst is
            # the bit-field layout  512*jh + 128*sU + 32*h + 16*f + 2*g + q
            TA = gp.tile([128, 4096], f32, tag="TA")
            t1_in = EL[:].rearrange("p (gq jh jl) -> p gq jh jl",
                                    gq=16, jh=8, jl=32)
            t1_out = TA[:].rearrange("p (jh sl gq) -> p gq jh sl",
                                     jh=8, sl=32, gq=16)
            nc.vector.transpose(t1_out, t1_in)
            # T2: pull (f, g, q) into partitions, jl back to free:
            # OUTp[32*i4 + 16f + 2g + q][128*jh + 4*jl + h]
            OUTp = gp.tile([128, 1024], f32, tag="OUTp")
            t2_in = TA[:].rearrange("p (jh sU h f gq) -> p jh h sU (f gq)",
                                    jh=8, sU=4, h=4, f=2, gq=16)[:, :, :, 0]
            t2_out = OUTp[:].rearrange("p (jh jl h) -> p jh h jl",
                                       jh=8, jl=32, h=4)
            nc.vector.transpose(t2_out, t2_in)
            # exp + row sums (free accumulate), normalize
            EX = gp.tile([128, 1024], f32, tag="EX")
            Z = gp.tile([128, 1], f32, tag="Z")
            nc.scalar.activation(EX[:], OUTp[:], AF.Exp, accum_out=Z[:])
            rz = gp.tile([128, 1], f32, tag="rz")
            nc.vector.reciprocal(rz[:], Z[:])
            OUTN = gp.tile([128, 1024], f32, tag="OUTN")
            nc.vector.tensor_scalar(OUTN[:], EX[:], rz[:], None,
                                    op0=bass.mybir.AluOpType.mult)
            for ii in range(4):
                for ff in range(2):
                    p0 = 32 * ii + 16 * ff
                    nc.sync.dma_start(hbmv[G, ii, ff], OUTN[p0:p0 + 16, :])
    nc.compile()
    return nc


def _host_inputs(x, W1, b1, W2, b2, W3, b3, core, n_i=IPC, act_dt=None):
    import ml_dtypes
    if act_dt is None:
        act_dt = ml_dtypes.bfloat16
    W1a, W1b = W1[:, :D], W1[:, D:]
    xT = np.ascontiguousarray(x.T).astype(np.float32)
    i0 = core * n_i
    W2bd = np.zeros((128, 128), np.float32)
    W2bd[:64, :64] = W2.T
    W2bd[64:, 64:] = W2.T
    W3bd = np.zeros((128, 32), np.float32)
    W3bd[:64, 0:8:2] = W3.T   # half0 -> rows m = 2h
    W3bd[64:, 1:8:2] = W3.T   # half1 -> rows m = 2h + 1
    b3pat = np.asarray([b3[((p % 32) // 2) % 4] for p in range(128)],
                       np.float32)
    cb = np.zeros((128, CB_COLS), np.float32)
    o = 0
    cb[0:64, o:o + 1024] = xT; o += 1024
    cb[0:64, o:o + n_i] = xT[:, i0:i0 + n_i]; o += n_i
    cb[0:64, o:o + 64] = W1a.T; o += 64
    cb[0:64, o:o + 64] = W1b.T; o += 64
    cb[:, o] = np.concatenate([b1, b1]); o += 1
    cb[:, o] = np.concatenate([b2, b2]); o += 1
    cb[:, o] = b3pat; o += 1
    cbh = np.zeros((128, 160), np.float32)
    cbh[:, 0:128] = W2bd
    cbh[:, 128:160] = W3bd
    return {"cb": cb, "cbh": cbh.astype(act_dt)}


def kernel(x, W1, b1, W2, b2, W3, b3):
    import sys
    if '/opt/trn_rl_repo' not in sys.path:
        sys.path.insert(0, '/opt/trn_rl_repo')
    from concourse.bass_utils import run_bass_kernel_spmd

    key = (IPC, "bfloat16")
    if key not in _PROG_CACHE:
        _PROG_CACHE[key] = _build_program(*key)
    nc = _PROG_CACHE[key]

    x = np.asarray(x, np.float32)
    in_maps = [
        _host_inputs(x, np.asarray(W1, np.float32), np.asarray(b1, np.float32),
                     np.asarray(W2, np.float32), np.asarray(b2, np.float32),
                     np.asarray(W3, np.float32), np.asarray(b3, np.float32), k)
        for k in range(NCORES)
    ]
    res = run_bass_kernel_spmd(nc, in_maps, list(range(NCORES)))
    rows = np.concatenate([res.results[k]["out"] for k in range(NCORES)], axis=0)
    return rows.reshape(HEADS, N, N)
